# revision 12
# baseline (speedup 1.0000x reference)
"""Trainium2 Bass kernel for nn_PhysJointModel (joint physics scan).

Physics: per (batch b, joint j), semi-implicit Euler over T steps:
    vt' = d*vt + nG_t*th + U_t        (vt = DT*dtheta)
    th' = th + vt'
with d = 1 - DT*damping/I (per j), nG_t = -DT^2*bk_t/I, U_t = DT^2*af_t/I.

Parallelization: pure data-parallel over batch across 8 cores; within a core,
time is parallelized by a chunked linear scan:
  pass A: per chunk (n_c chunks of C steps), evolve 2 homogeneous bases
          (X=(1,0),(0,1)) + the particular solution (X=(0,0)) -> chunk map
          X_out = H_c X_in + P_c
  pass B: chain the chunk maps sequentially (hierarchical: groups) to get
          per-chunk initial states
  pass C: replay the recurrence per chunk with true inits, emitting theta.
All heavy ops are [128, 512] fp32 tensor_tensor on DVE with a static share
on GPSIMD (they do not contend: fp32 TT never takes the shared SBUF port).
"""

import os
import sys
from contextlib import ExitStack

import numpy as np

sys.path.insert(0, "/opt/trn_rl_repo")

SR = 60
DT = 1.0 / SR
J = 8
BL = 128          # batch rows per core
N_CORES = 8

f32 = np.float32


# ---------------------------------------------------------------------------
# device kernel builder
# ---------------------------------------------------------------------------

def build_kernel(T=2048, n_c=64, C=32, tblk=64, debug=False, use_gpsimd=True):
    """Build and compile the per-core Bass module. Returns (nc, names)."""
    import concourse.bass as bass
    import concourse.tile as tile
    from concourse import bacc, mybir

    assert n_c * C == T
    FREE = n_c * J                     # free width of state tiles
    n_xt = T // tblk                   # number of x tiles
    GRP = 8                            # pass-B groups
    LNK = n_c // GRP                   # links per group
    dt = mybir.dt.float32
    mult = mybir.AluOpType.mult
    add = mybir.AluOpType.add

    nc = bacc.Bacc("TRN2", target_bir_lowering=False, debug=debug)
    gps = nc.gpsimd if use_gpsimd else nc.vector

    xs = nc.dram_tensor("xs", [BL, T * 32], dt, kind="ExternalInput").ap()
    st_th = nc.dram_tensor("st_th", [BL, J], dt, kind="ExternalInput").ap()
    st_vt = nc.dram_tensor("st_vt", [BL, J], dt, kind="ExternalInput").ap()
    # weight tiles, host-prebroadcast:
    #  wf0/wf1/wk0n/wk1n: [BL, tblk*J] value w[j] repeated per t
    #  d3: [BL, n_c*J] value d[j] repeated per chunk
    wf0 = nc.dram_tensor("wf0", [BL, tblk * J], dt, kind="ExternalInput").ap()
    wf1 = nc.dram_tensor("wf1", [BL, tblk * J], dt, kind="ExternalInput").ap()
    wk0n = nc.dram_tensor("wk0n", [BL, tblk * J], dt, kind="ExternalInput").ap()
    wk1n = nc.dram_tensor("wk1n", [BL, tblk * J], dt, kind="ExternalInput").ap()
    d3 = nc.dram_tensor("d3", [BL, FREE], dt, kind="ExternalInput").ap()

    out_d = nc.dram_tensor("out_d", [BL, T * J], dt, kind="ExternalOutput").ap()
    vt_d = nc.dram_tensor("vt_d", [BL, J], dt, kind="ExternalOutput").ap()

    with tile.TileContext(nc) as tc, ExitStack() as ctx:
        big = ctx.enter_context(tc.tile_pool(name="big", bufs=1))
        xpool = ctx.enter_context(tc.tile_pool(name="xp", bufs=2))
        wpool = ctx.enter_context(tc.tile_pool(name="wp", bufs=1))
        spool = ctx.enter_context(tc.tile_pool(name="sp", bufs=2))
        tpool = ctx.enter_context(tc.tile_pool(name="tp", bufs=1))
        bpool = ctx.enter_context(tc.tile_pool(name="bp", bufs=1))

        # persistent buffers
    # nG/U laid out [b, (t j)] t-major; step-s slice of chunk set is strided
        nG_b = big.tile([BL, T * J], dt, tag="nG")
        U_b = big.tile([BL, T * J], dt, tag="U")

        w_f0 = wpool.tile([BL, tblk * J], dt, tag="wf0")
        w_f1 = wpool.tile([BL, tblk * J], dt, tag="wf1")
        w_k0 = wpool.tile([BL, tblk * J], dt, tag="wk0")
        w_k1 = wpool.tile([BL, tblk * J], dt, tag="wk1")
        d3_t = wpool.tile([BL, FREE], dt, tag="d3")
        nc.sync.dma_start(w_f0[:], wf0)
        nc.sync.dma_start(w_f1[:], wf1)
        nc.sync.dma_start(w_k0[:], wk0n)
        nc.sync.dma_start(w_k1[:], wk1n)
        nc.sync.dma_start(d3_t[:], d3)

        st_th_t = wpool.tile([BL, J], dt, tag="stth")
        st_vt_t = wpool.tile([BL, J], dt, tag="stvt")
        nc.sync.dma_start(st_th_t[:], st_th)
        nc.sync.dma_start(st_vt_t[:], st_vt)

        # ---------------- precompute nG / U ----------------
        wf0v = w_f0[:].rearrange("p (t j) -> p t j", j=J)
        wf1v = w_f1[:].rearrange("p (t j) -> p t j", j=J)
        wk0v = w_k0[:].rearrange("p (t j) -> p t j", j=J)
        wk1v = w_k1[:].rearrange("p (t j) -> p t j", j=J)
        nGv = nG_b[:].rearrange("p (t j) -> p t j", j=J)
        Uv = U_b[:].rearrange("p (t j) -> p t j", j=J)

        for it in range(n_xt):
            xt = xpool.tile([BL, tblk * 32], dt, tag="xt")
            nc.sync.dma_start(xt[:], xs[:, it * tblk * 32:(it + 1) * tblk * 32])
            xv = xt[:].rearrange("p (t j k) -> p t j k", j=J, k=4)
            ot = slice(it * tblk, (it + 1) * tblk)

            t0 = tpool.tile([BL, tblk * J], dt, tag="mmA")
            t0v = t0[:].rearrange("p (t j) -> p t j", j=J)
            t1 = tpool.tile([BL, tblk * J], dt, tag="mmB")
            t1v = t1[:].rearrange("p (t j) -> p t j", j=J)
            nc.vector.tensor_tensor(t0v, xv[:, :, :, 0], wf0v, op=mult)
            gps.tensor_tensor(t1v, xv[:, :, :, 1], wf1v, op=mult)
            nc.vector.tensor_tensor(Uv[:, ot, :], t0v, t1v, op=add)

            t2 = tpool.tile([BL, tblk * J], dt, tag="vdA")
            t2v = t2[:].rearrange("p (t j) -> p t j", j=J)
            t3 = tpool.tile([BL, tblk * J], dt, tag="vdB")
            t3v = t3[:].rearrange("p (t j) -> p t j", j=J)
            nc.vector.tensor_tensor(t2v, xv[:, :, :, 2], wk0v, op=mult)
            gps.tensor_tensor(t3v, xv[:, :, :, 3], wk1v, op=mult)
            nc.vector.tensor_tensor(nGv[:, ot, :], t2v, t3v, op=add)

        # chunk-sliced views for pass A/C: [p, c, s, j] -> step slice [p, c, j]
        nGs = nG_b[:].rearrange("p (c s j) -> p c s j", s=C, j=J)
        Us = U_b[:].rearrange("p (c s j) -> p c s j", s=C, j=J)
        d3v = d3_t[:].rearrange("p (c j) -> p c j", j=J)

        # ---------------- pass A ----------------
        def st(tag, fill=None):
            t_ = spool.tile([BL, FREE], dt, tag=tag)
            if fill is not None:
                nc.vector.memset(t_[:], fill)
            return t_

        thA, vtA = st("thA", 1.0), st("vtA", 0.0)
        thB, vtB = st("thB", 0.0), st("vtB", 1.0)
        thP, vtP = st("thP", 0.0), st("vtP", 0.0)

        def cv(t_):
            return t_[:].rearrange("p (c j) -> p c j", j=J)

        for s in range(C):
            g = nGs[:, :, s, :]
            # basis A (DVE)
            mmA = tpool.tile([BL, FREE], dt, tag="mmA")
            vdA = tpool.tile([BL, FREE], dt, tag="vdA")
            nc.vector.tensor_tensor(cv(mmA), g, cv(thA), op=mult)
            nc.vector.tensor_tensor(cv(vdA), d3v, cv(vtA), op=mult)
            vtA_n = spool.tile([BL, FREE], dt, tag="vtA")
            nc.vector.tensor_tensor(vtA_n[:], vdA[:], mmA[:], op=add)
            thA_n = spool.tile([BL, FREE], dt, tag="thA")
            nc.vector.tensor_tensor(thA_n[:], thA[:], vtA_n[:], op=add)
            # basis B (GPSIMD)
            mmB = tpool.tile([BL, FREE], dt, tag="mmB")
            vdB = tpool.tile([BL, FREE], dt, tag="vdB")
            gps.tensor_tensor(cv(mmB), g, cv(thB), op=mult)
            gps.tensor_tensor(cv(vdB), d3v, cv(vtB), op=mult)
            vtB_n = spool.tile([BL, FREE], dt, tag="vtB")
            gps.tensor_tensor(vtB_n[:], vdB[:], mmB[:], op=add)
            thB_n = spool.tile([BL, FREE], dt, tag="thB")
            gps.tensor_tensor(thB_n[:], thB[:], vtB_n[:], op=add)
            # particular (DVE)
            mmP = tpool.tile([BL, FREE], dt, tag="mmP")
            vdP = tpool.tile([BL, FREE], dt, tag="vdP")
            nc.vector.tensor_tensor(cv(mmP), g, cv(thP), op=mult)
            nc.vector.tensor_tensor(cv(vdP), d3v, cv(vtP), op=mult)
            sv = tpool.tile([BL, FREE], dt, tag="sv")
            nc.vector.tensor_tensor(sv[:], vdP[:], mmP[:], op=add)
            vtP_n = spool.tile([BL, FREE], dt, tag="vtP")
            nc.vector.tensor_tensor(cv(vtP_n), sv[:].rearrange("p (c j) -> p c j", j=J), Us[:, :, s, :], op=add)
            thP_n = spool.tile([BL, FREE], dt, tag="thP")
            nc.vector.tensor_tensor(thP_n[:], thP[:], vtP_n[:], op=add)

            thA, vtA, thB, vtB, thP, vtP = thA_n, vtA_n, thB_n, vtB_n, thP_n, vtP_n

        # chunk maps: H = [[thA, thB], [vtA, vtB]], P = (thP, vtP)
        # ---------------- pass B ----------------
        # group-compose: cumulative affine map per group over its LNK links
        # map tiles viewed [p, g, l, j]
        def gv(t_):
            return t_[:].rearrange("p (g l j) -> p g l j", l=LNK, j=J)

        H = {"h11": thA, "h12": thB, "h21": vtA, "h22": vtB, "p1": thP, "p2": vtP}
        Hg = {k: gv(v) for k, v in H.items()}

        # cumulative maps per group [p, g, j]
        def new_cm():
            cm_ = {k: bpool.tile([BL, GRP * J], dt, tag="cm" + k, name="cm" + k,
                                 bufs=2) for k in H}
            return cm_, {k: cm_[k][:].rearrange("p (g j) -> p g j", j=J) for k in cm_}

        cm, cmv = new_cm()
        for k in H:
            nc.vector.tensor_copy(cmv[k], Hg[k][:, :, 0, :])

        def compose(dst, lnk, src, tmp_tag):
            """dst = lnk o src (2x2 affine compose), all dicts of [p,g,j] APs.
            lnk entries: APs; src/dst: APs. Uses temps from tpool."""
            ts_ = {}
            for nm in ("a", "b", "c", "d2", "e", "f2"):
                tt = tpool.tile([BL, GRP * J], dt, tag=tmp_tag + nm)
                ts_[nm] = tt[:].rearrange("p (g j) -> p g j", j=J)
            # new_h11 = l11*s11 + l12*s21 ; new_h21 = l21*s11 + l22*s21
            nc.vector.tensor_tensor(ts_["a"], lnk["h11"], src["h11"], op=mult)
            nc.vector.tensor_tensor(ts_["b"], lnk["h12"], src["h21"], op=mult)
            gps.tensor_tensor(ts_["c"], lnk["h21"], src["h11"], op=mult)
            gps.tensor_tensor(ts_["d2"], lnk["h22"], src["h21"], op=mult)
            nc.vector.tensor_tensor(dst["h11"], ts_["a"], ts_["b"], op=add)
            gps.tensor_tensor(dst["h21"], ts_["c"], ts_["d2"], op=add)
            # new_h12 = l11*s12 + l12*s22 ; new_h22 = l21*s12 + l22*s22
            nc.vector.tensor_tensor(ts_["a"], lnk["h11"], src["h12"], op=mult)
            nc.vector.tensor_tensor(ts_["b"], lnk["h12"], src["h22"], op=mult)
            gps.tensor_tensor(ts_["c"], lnk["h21"], src["h12"], op=mult)
            gps.tensor_tensor(ts_["d2"], lnk["h22"], src["h22"], op=mult)
            nc.vector.tensor_tensor(dst["h12"], ts_["a"], ts_["b"], op=add)
            gps.tensor_tensor(dst["h22"], ts_["c"], ts_["d2"], op=add)
            # new_p = l(H)@src_p + l(p):  p1' = l11*sp1 + l12*sp2 + lp1
            nc.vector.tensor_tensor(ts_["e"], lnk["h11"], src["p1"], op=mult)
            nc.vector.tensor_tensor(ts_["f2"], lnk["h12"], src["p2"], op=mult)
            nc.vector.tensor_tensor(ts_["a"], ts_["e"], ts_["f2"], op=add)
            nc.vector.tensor_tensor(dst["p1"], ts_["a"], lnk["p1"], op=add)
            gps.tensor_tensor(ts_["e"], lnk["h21"], src["p1"], op=mult)
            gps.tensor_tensor(ts_["f2"], lnk["h22"], src["p2"], op=mult)
            gps.tensor_tensor(ts_["b"], ts_["e"], ts_["f2"], op=add)
            gps.tensor_tensor(dst["p2"], ts_["b"], lnk["p2"], op=add)

        for l in range(1, LNK):
            lnk = {k: Hg[k][:, :, l, :] for k in H}
            cm_n, cmv_n = new_cm()
            compose(cmv_n, lnk, cmv, "gc")
            cm, cmv = cm_n, cmv_n

        # chain group-start states sequentially: Xg tiles [p, (g+1)*J]... store
        # group-start states into the init tiles directly.
        X1i = bpool.tile([BL, FREE], dt, tag="X1i")
        X2i = bpool.tile([BL, FREE], dt, tag="X2i")
        X1g = X1i[:].rearrange("p (g l j) -> p g l j", l=LNK, j=J)
        X2g = X2i[:].rearrange("p (g l j) -> p g l j", l=LNK, j=J)
        # X_{0,0} = (st_th, st_vt)
        nc.vector.tensor_copy(X1g[:, 0, 0, :], st_th_t[:])
        nc.vector.tensor_copy(X2g[:, 0, 0, :], st_vt_t[:])
        for g_ in range(GRP - 1):
            a_ = tpool.tile([BL, J], dt, tag="cha")
            b_ = tpool.tile([BL, J], dt, tag="chb")
            c_ = tpool.tile([BL, J], dt, tag="chc")
            nc.vector.tensor_tensor(a_[:], cmv["h11"][:, g_, :], X1g[:, g_, 0, :], op=mult)
            nc.vector.tensor_tensor(b_[:], cmv["h12"][:, g_, :], X2g[:, g_, 0, :], op=mult)
            nc.vector.tensor_tensor(c_[:], a_[:], b_[:], op=add)
            nc.vector.tensor_tensor(X1g[:, g_ + 1, 0, :], c_[:], cmv["p1"][:, g_, :], op=add)
            a2_ = tpool.tile([BL, J], dt, tag="cha")
            b2_ = tpool.tile([BL, J], dt, tag="chb")
            c2_ = tpool.tile([BL, J], dt, tag="chc")
            nc.vector.tensor_tensor(a2_[:], cmv["h21"][:, g_, :], X1g[:, g_, 0, :], op=mult)
            nc.vector.tensor_tensor(b2_[:], cmv["h22"][:, g_, :], X2g[:, g_, 0, :], op=mult)
            nc.vector.tensor_tensor(c2_[:], a2_[:], b2_[:], op=add)
            nc.vector.tensor_tensor(X2g[:, g_ + 1, 0, :], c2_[:], cmv["p2"][:, g_, :], op=add)

        # recover within-group inits: X_{g,l} = link_{l-1}(X_{g,l-1}), vec over g
        for l in range(1, LNK):
            lnk = {k: Hg[k][:, :, l - 1, :] for k in H}
            a_ = tpool.tile([BL, GRP * J], dt, tag="rca")
            av = a_[:].rearrange("p (g j) -> p g j", j=J)
            b_ = tpool.tile([BL, GRP * J], dt, tag="rcb")
            bv = b_[:].rearrange("p (g j) -> p g j", j=J)
            nc.vector.tensor_tensor(av, lnk["h11"], X1g[:, :, l - 1, :], op=mult)
            nc.vector.tensor_tensor(bv, lnk["h12"], X2g[:, :, l - 1, :], op=mult)
            e_ = tpool.tile([BL, GRP * J], dt, tag="rce")
            ev2 = e_[:].rearrange("p (g j) -> p g j", j=J)
            nc.vector.tensor_tensor(ev2, av, bv, op=add)
            c_ = tpool.tile([BL, GRP * J], dt, tag="rcc")
            cv2 = c_[:].rearrange("p (g j) -> p g j", j=J)
            d_ = tpool.tile([BL, GRP * J], dt, tag="rcd")
            dv2 = d_[:].rearrange("p (g j) -> p g j", j=J)
            f_ = tpool.tile([BL, GRP * J], dt, tag="rcf")
            fv2 = f_[:].rearrange("p (g j) -> p g j", j=J)
            gps.tensor_tensor(cv2, lnk["h21"], X1g[:, :, l - 1, :], op=mult)
            gps.tensor_tensor(dv2, lnk["h22"], X2g[:, :, l - 1, :], op=mult)
            gps.tensor_tensor(fv2, cv2, dv2, op=add)
            nc.vector.tensor_tensor(X1g[:, :, l, :], ev2, lnk["p1"], op=add)
            gps.tensor_tensor(X2g[:, :, l, :], fv2, lnk["p2"], op=add)

        # ---------------- pass C ----------------
        # th threaded as [p, c, j] APs; th' written straight into the
        # just-consumed U slice, which turns U_b into the output buffer.
        th_ap = X1i[:].rearrange("p (c j) -> p c j", j=J)
        vt_ap = X2i[:].rearrange("p (c j) -> p c j", j=J)
        outv = U_b[:].rearrange("p (c s j) -> p c s j", s=C, j=J)
        for s in range(C):
            g = nGs[:, :, s, :]
            mm = tpool.tile([BL, FREE], dt, tag="mmP")
            gps.tensor_tensor(cv(mm), g, th_ap, op=mult)
            vd = tpool.tile([BL, FREE], dt, tag="vdP")
            nc.vector.tensor_tensor(cv(vd), d3v, vt_ap, op=mult)
            vt_n = spool.tile([BL, FREE], dt, tag="thB")
            nc.vector.tensor_tensor(vt_n[:], vd[:], mm[:], op=add)
            vt2 = spool.tile([BL, FREE], dt, tag="vtB")
            nc.vector.tensor_tensor(cv(vt2), cv(vt_n), Us[:, :, s, :], op=add)
            nc.vector.tensor_tensor(outv[:, :, s, :], th_ap, cv(vt2), op=add)
            th_ap = outv[:, :, s, :]
            vt_ap = cv(vt2)

        # final dtheta: vt of last chunk * SR
        vt_last = vt_ap[:, n_c - 1, :]
        vts = tpool.tile([BL, J], dt, tag="vts")
        nc.vector.tensor_scalar_mul(vts[:], vt_last, float(SR))
        nc.sync.dma_start(vt_d, vts[:])

        # out DMA: U_b now holds theta outputs in (t j) layout
        half = T * J // 2
        nc.sync.dma_start(out_d[:, :half], U_b[:, :half])
        nc.sync.dma_start(out_d[:, half:], U_b[:, half:])

    nc.compile()
    return nc


# ---------------------------------------------------------------------------
# host glue
# ---------------------------------------------------------------------------

def _host_weights(M, inertia, damping, T, n_c, C, tblk):
    d64 = 1.0 - DT * damping.astype(np.float64) / inertia.astype(np.float64)
    DT2_I = DT * DT / inertia.astype(np.float64)
    M64 = M.astype(np.float64)
    wf = (DT2_I[:, None] * M64).astype(f32)          # [J,2]
    nwk = (-DT2_I[:, None] * M64 ** 2).astype(f32)   # [J,2]
    d_ = d64.astype(f32)

    def rep(vecj, n):
        return np.broadcast_to(np.tile(vecj, n)[None, :], (BL, n * J)).copy()

    return {
        "wf0": rep(wf[:, 0], tblk),
        "wf1": rep(wf[:, 1], tblk),
        "wk0n": rep(nwk[:, 0], tblk),
        "wk1n": rep(nwk[:, 1], tblk),
        "d3": rep(d_, n_c),
    }


_CACHE = {}


def _get_kernel(T=2048, n_c=64, C=32, tblk=64):
    key = (T, n_c, C, tblk)
    if key not in _CACHE:
        _CACHE[key] = build_kernel(T, n_c, C, tblk)
    return _CACHE[key]


def kernel(x, state0, M, inertia, damping):
    from concourse import bass_utils

    x = np.ascontiguousarray(x, dtype=f32)
    state0 = np.ascontiguousarray(state0, dtype=f32)
    M = np.asarray(M, f32)
    inertia = np.asarray(inertia, f32)
    damping = np.asarray(damping, f32)

    B, T, _ = x.shape
    n_c, C, tblk = 64, 32, 64
    nc = _get_kernel(T, n_c, C, tblk)
    w = _host_weights(M, inertia, damping, T, n_c, C, tblk)

    in_maps = []
    for k in range(N_CORES):
        sl = slice(k * BL, (k + 1) * BL)
        in_maps.append({
            "xs": x[sl].reshape(BL, T * 32),
            "st_th": np.ascontiguousarray(state0[sl, :, 0]),
            "st_vt": np.ascontiguousarray(state0[sl, :, 1] * f32(DT)),
            **w,
        })

    res = bass_utils.run_bass_kernel_spmd(nc, in_maps, core_ids=list(range(N_CORES)))

    out = np.empty((B, T, J), f32)
    v_T = np.empty((B, J), f32)
    for k in range(N_CORES):
        sl = slice(k * BL, (k + 1) * BL)
        out[sl] = res.results[k]["out_d"].reshape(BL, T, J)
        v_T[sl] = res.results[k]["vt_d"]

    th_T = out[:, T - 1, :]
    final_state = np.stack([th_T, v_T], axis=2)
    muscle_states = final_state[..., None] * M[None, :, None, :]
    return out, muscle_states, final_state


# revision 22
# speedup vs baseline: 1.0871x; 1.0871x over previous
"""Trainium2 Bass kernel for nn_PhysJointModel (joint physics scan).

Physics: per (batch b, joint j), semi-implicit Euler over T steps:
    vt' = d*vt + nG_t*th + U_t        (vt = DT*dtheta)
    th' = th + vt'
with d = 1 - DT*damping/I (per j), nG_t = -DT^2*bk_t/I, U_t = DT^2*af_t/I.

Parallelization: pure data-parallel over batch across 8 cores; within a core,
time is parallelized by a chunked linear scan:
  pass A: per chunk (n_c chunks of C steps), evolve 2 homogeneous bases
          (X=(1,0),(0,1)) + the particular solution (X=(0,0)) -> chunk map
          X_out = H_c X_in + P_c
  pass B: chain the chunk maps sequentially (hierarchical: groups) to get
          per-chunk initial states
  pass C: replay the recurrence per chunk with true inits, emitting theta.
All heavy ops are [128, 512] fp32 tensor_tensor on DVE with a static share
on GPSIMD (they do not contend: fp32 TT never takes the shared SBUF port).
"""

import os
import sys
from contextlib import ExitStack

import numpy as np

sys.path.insert(0, "/opt/trn_rl_repo")

SR = 60
DT = 1.0 / SR
J = 8
BL = 128          # batch rows per core
N_CORES = 8

f32 = np.float32


# ---------------------------------------------------------------------------
# device kernel builder
# ---------------------------------------------------------------------------

def build_kernel(T=2048, n_c=64, C=32, tblk=64, debug=False, use_gpsimd=True, gps_passc=False):
    """Build and compile the per-core Bass module. Returns (nc, names)."""
    import concourse.bass as bass
    import concourse.tile as tile
    from concourse import bacc, mybir

    assert n_c * C == T
    FREE = n_c * J                     # free width of state tiles
    n_xt = T // tblk                   # number of x tiles
    GRP = 8                            # pass-B groups
    LNK = n_c // GRP                   # links per group
    dt = mybir.dt.float32
    mult = mybir.AluOpType.mult
    add = mybir.AluOpType.add

    nc = bacc.Bacc("TRN2", target_bir_lowering=False, debug=debug)
    gps = nc.gpsimd if use_gpsimd else nc.vector

    xs = nc.dram_tensor("xs", [BL, T * 32], dt, kind="ExternalInput").ap()
    st_th = nc.dram_tensor("st_th", [BL, J], dt, kind="ExternalInput").ap()
    st_vt = nc.dram_tensor("st_vt", [BL, J], dt, kind="ExternalInput").ap()
    # weight tiles, host-prebroadcast:
    #  wf0/wf1/wk0n/wk1n: [BL, tblk*J] value w[j] repeated per t
    #  d3: [BL, n_c*J] value d[j] repeated per chunk
    wf0 = nc.dram_tensor("wf0", [BL, tblk * J], dt, kind="ExternalInput").ap()
    wf1 = nc.dram_tensor("wf1", [BL, tblk * J], dt, kind="ExternalInput").ap()
    wk0n = nc.dram_tensor("wk0n", [BL, tblk * J], dt, kind="ExternalInput").ap()
    wk1n = nc.dram_tensor("wk1n", [BL, tblk * J], dt, kind="ExternalInput").ap()
    d3 = nc.dram_tensor("d3", [BL, FREE], dt, kind="ExternalInput").ap()

    out_d = nc.dram_tensor("out_d", [BL, T * J], dt, kind="ExternalOutput").ap()
    vt_d = nc.dram_tensor("vt_d", [BL, J], dt, kind="ExternalOutput").ap()

    with tile.TileContext(nc) as tc, ExitStack() as ctx:
        big = ctx.enter_context(tc.tile_pool(name="big", bufs=1))
        xpool = ctx.enter_context(tc.tile_pool(name="xp", bufs=2))
        wpool = ctx.enter_context(tc.tile_pool(name="wp", bufs=1))
        spool = ctx.enter_context(tc.tile_pool(name="sp", bufs=2))
        tpool = ctx.enter_context(tc.tile_pool(name="tp", bufs=1))
        bpool = ctx.enter_context(tc.tile_pool(name="bp", bufs=1))

        # persistent buffers
    # nG/U laid out [b, (t j)] t-major; step-s slice of chunk set is strided
        nG_b = big.tile([BL, T * J], dt, tag="nG")
        U_b = big.tile([BL, T * J], dt, tag="U")

        w_f0 = wpool.tile([BL, tblk * J], dt, tag="wf0")
        w_f1 = wpool.tile([BL, tblk * J], dt, tag="wf1")
        w_k0 = wpool.tile([BL, tblk * J], dt, tag="wk0")
        w_k1 = wpool.tile([BL, tblk * J], dt, tag="wk1")
        d3_t = wpool.tile([BL, FREE], dt, tag="d3")
        nc.sync.dma_start(w_f0[:], wf0)
        nc.sync.dma_start(w_f1[:], wf1)
        nc.sync.dma_start(w_k0[:], wk0n)
        nc.sync.dma_start(w_k1[:], wk1n)
        nc.sync.dma_start(d3_t[:], d3)

        st_th_t = wpool.tile([BL, J], dt, tag="stth")
        st_vt_t = wpool.tile([BL, J], dt, tag="stvt")
        nc.sync.dma_start(st_th_t[:], st_th)
        nc.sync.dma_start(st_vt_t[:], st_vt)

        # ---------------- precompute nG / U ----------------
        wf0v = w_f0[:].rearrange("p (t j) -> p t j", j=J)
        wf1v = w_f1[:].rearrange("p (t j) -> p t j", j=J)
        wk0v = w_k0[:].rearrange("p (t j) -> p t j", j=J)
        wk1v = w_k1[:].rearrange("p (t j) -> p t j", j=J)
        nGv = nG_b[:].rearrange("p (t j) -> p t j", j=J)
        Uv = U_b[:].rearrange("p (t j) -> p t j", j=J)

        for it in range(n_xt):
            xt = xpool.tile([BL, tblk * 32], dt, tag="xt")
            nc.sync.dma_start(xt[:], xs[:, it * tblk * 32:(it + 1) * tblk * 32])
            xv = xt[:].rearrange("p (t j k) -> p t j k", j=J, k=4)
            ot = slice(it * tblk, (it + 1) * tblk)

            t0 = tpool.tile([BL, tblk * J], dt, tag="mmA")
            t0v = t0[:].rearrange("p (t j) -> p t j", j=J)
            t1 = tpool.tile([BL, tblk * J], dt, tag="mmB")
            t1v = t1[:].rearrange("p (t j) -> p t j", j=J)
            nc.vector.tensor_tensor(t0v, xv[:, :, :, 0], wf0v, op=mult)
            gps.tensor_tensor(t1v, xv[:, :, :, 1], wf1v, op=mult)
            nc.vector.tensor_tensor(Uv[:, ot, :], t0v, t1v, op=add)

            t2 = tpool.tile([BL, tblk * J], dt, tag="vdA")
            t2v = t2[:].rearrange("p (t j) -> p t j", j=J)
            t3 = tpool.tile([BL, tblk * J], dt, tag="vdB")
            t3v = t3[:].rearrange("p (t j) -> p t j", j=J)
            nc.vector.tensor_tensor(t2v, xv[:, :, :, 2], wk0v, op=mult)
            gps.tensor_tensor(t3v, xv[:, :, :, 3], wk1v, op=mult)
            nc.vector.tensor_tensor(nGv[:, ot, :], t2v, t3v, op=add)

        # chunk-sliced views for pass A/C: [p, c, s, j] -> step slice [p, c, j]
        nGs = nG_b[:].rearrange("p (c s j) -> p c s j", s=C, j=J)
        Us = U_b[:].rearrange("p (c s j) -> p c s j", s=C, j=J)
        d3v = d3_t[:].rearrange("p (c j) -> p c j", j=J)

        # ---------------- pass A ----------------
        def st(tag, fill=None):
            t_ = spool.tile([BL, FREE], dt, tag=tag)
            if fill is not None:
                nc.vector.memset(t_[:], fill)
            return t_

        thA, vtA = st("thA", 1.0), st("vtA", 0.0)
        thB, vtB = st("thB", 0.0), st("vtB", 1.0)
        thP, vtP = st("thP", 0.0), st("vtP", 0.0)

        def cv(t_):
            return t_[:].rearrange("p (c j) -> p c j", j=J)

        for s in range(C):
            g = nGs[:, :, s, :]
            # basis A (DVE)
            mmA = tpool.tile([BL, FREE], dt, tag="mmA")
            vdA = tpool.tile([BL, FREE], dt, tag="vdA")
            nc.vector.tensor_tensor(cv(mmA), g, cv(thA), op=mult)
            nc.vector.tensor_tensor(cv(vdA), d3v, cv(vtA), op=mult)
            vtA_n = spool.tile([BL, FREE], dt, tag="vtA")
            nc.vector.tensor_tensor(vtA_n[:], vdA[:], mmA[:], op=add)
            thA_n = spool.tile([BL, FREE], dt, tag="thA")
            nc.vector.tensor_tensor(thA_n[:], thA[:], vtA_n[:], op=add)
            # basis B (GPSIMD)
            mmB = tpool.tile([BL, FREE], dt, tag="mmB")
            vdB = tpool.tile([BL, FREE], dt, tag="vdB")
            gps.tensor_tensor(cv(mmB), g, cv(thB), op=mult)
            gps.tensor_tensor(cv(vdB), d3v, cv(vtB), op=mult)
            vtB_n = spool.tile([BL, FREE], dt, tag="vtB")
            gps.tensor_tensor(vtB_n[:], vdB[:], mmB[:], op=add)
            thB_n = spool.tile([BL, FREE], dt, tag="thB")
            gps.tensor_tensor(thB_n[:], thB[:], vtB_n[:], op=add)
            # particular (DVE)
            mmP = tpool.tile([BL, FREE], dt, tag="mmP")
            vdP = tpool.tile([BL, FREE], dt, tag="vdP")
            nc.vector.tensor_tensor(cv(mmP), g, cv(thP), op=mult)
            nc.vector.tensor_tensor(cv(vdP), d3v, cv(vtP), op=mult)
            sv = tpool.tile([BL, FREE], dt, tag="sv")
            nc.vector.tensor_tensor(sv[:], vdP[:], mmP[:], op=add)
            vtP_n = spool.tile([BL, FREE], dt, tag="vtP")
            nc.vector.tensor_tensor(cv(vtP_n), sv[:].rearrange("p (c j) -> p c j", j=J), Us[:, :, s, :], op=add)
            thP_n = spool.tile([BL, FREE], dt, tag="thP")
            nc.vector.tensor_tensor(thP_n[:], thP[:], vtP_n[:], op=add)

            thA, vtA, thB, vtB, thP, vtP = thA_n, vtA_n, thB_n, vtB_n, thP_n, vtP_n

        # chunk maps: H = [[thA, thB], [vtA, vtB]], P = (thP, vtP)
        # ---------------- pass B ----------------
        # group-compose: cumulative affine map per group over its LNK links
        # map tiles viewed [p, g, l, j]
        def gv(t_):
            return t_[:].rearrange("p (g l j) -> p g l j", l=LNK, j=J)

        H = {"h11": thA, "h12": thB, "h21": vtA, "h22": vtB, "p1": thP, "p2": vtP}
        Hg = {k: gv(v) for k, v in H.items()}

        # cumulative maps per group [p, g, j]
        def new_cm():
            cm_ = {k: bpool.tile([BL, GRP * J], dt, tag="cm" + k, name="cm" + k,
                                 bufs=2) for k in H}
            return cm_, {k: cm_[k][:].rearrange("p (g j) -> p g j", j=J) for k in cm_}

        cm, cmv = new_cm()
        for k in H:
            nc.vector.tensor_copy(cmv[k], Hg[k][:, :, 0, :])

        def compose(dst, lnk, src, tmp_tag):
            """dst = lnk o src (2x2 affine compose), all dicts of [p,g,j] APs.
            lnk entries: APs; src/dst: APs. Uses temps from tpool."""
            ts_ = {}
            for nm in ("a", "b", "c", "d2", "e", "f2"):
                tt = tpool.tile([BL, GRP * J], dt, tag=tmp_tag + nm)
                ts_[nm] = tt[:].rearrange("p (g j) -> p g j", j=J)
            # new_h11 = l11*s11 + l12*s21 ; new_h21 = l21*s11 + l22*s21
            nc.vector.tensor_tensor(ts_["a"], lnk["h11"], src["h11"], op=mult)
            nc.vector.tensor_tensor(ts_["b"], lnk["h12"], src["h21"], op=mult)
            gps.tensor_tensor(ts_["c"], lnk["h21"], src["h11"], op=mult)
            gps.tensor_tensor(ts_["d2"], lnk["h22"], src["h21"], op=mult)
            nc.vector.tensor_tensor(dst["h11"], ts_["a"], ts_["b"], op=add)
            gps.tensor_tensor(dst["h21"], ts_["c"], ts_["d2"], op=add)
            # new_h12 = l11*s12 + l12*s22 ; new_h22 = l21*s12 + l22*s22
            nc.vector.tensor_tensor(ts_["a"], lnk["h11"], src["h12"], op=mult)
            nc.vector.tensor_tensor(ts_["b"], lnk["h12"], src["h22"], op=mult)
            gps.tensor_tensor(ts_["c"], lnk["h21"], src["h12"], op=mult)
            gps.tensor_tensor(ts_["d2"], lnk["h22"], src["h22"], op=mult)
            nc.vector.tensor_tensor(dst["h12"], ts_["a"], ts_["b"], op=add)
            gps.tensor_tensor(dst["h22"], ts_["c"], ts_["d2"], op=add)
            # new_p = l(H)@src_p + l(p):  p1' = l11*sp1 + l12*sp2 + lp1
            nc.vector.tensor_tensor(ts_["e"], lnk["h11"], src["p1"], op=mult)
            nc.vector.tensor_tensor(ts_["f2"], lnk["h12"], src["p2"], op=mult)
            nc.vector.tensor_tensor(ts_["a"], ts_["e"], ts_["f2"], op=add)
            nc.vector.tensor_tensor(dst["p1"], ts_["a"], lnk["p1"], op=add)
            gps.tensor_tensor(ts_["e"], lnk["h21"], src["p1"], op=mult)
            gps.tensor_tensor(ts_["f2"], lnk["h22"], src["p2"], op=mult)
            gps.tensor_tensor(ts_["b"], ts_["e"], ts_["f2"], op=add)
            gps.tensor_tensor(dst["p2"], ts_["b"], lnk["p2"], op=add)

        for l in range(1, LNK):
            lnk = {k: Hg[k][:, :, l, :] for k in H}
            cm_n, cmv_n = new_cm()
            compose(cmv_n, lnk, cmv, "gc")
            cm, cmv = cm_n, cmv_n

        # chain group-start states sequentially: Xg tiles [p, (g+1)*J]... store
        # group-start states into the init tiles directly.
        X1i = bpool.tile([BL, FREE], dt, tag="X1i")
        X2i = bpool.tile([BL, FREE], dt, tag="X2i")
        X1g = X1i[:].rearrange("p (g l j) -> p g l j", l=LNK, j=J)
        X2g = X2i[:].rearrange("p (g l j) -> p g l j", l=LNK, j=J)
        # X_{0,0} = (st_th, st_vt)
        nc.vector.tensor_copy(X1g[:, 0, 0, :], st_th_t[:])
        nc.vector.tensor_copy(X2g[:, 0, 0, :], st_vt_t[:])
        for g_ in range(GRP - 1):
            a_ = tpool.tile([BL, J], dt, tag="cha")
            b_ = tpool.tile([BL, J], dt, tag="chb")
            c_ = tpool.tile([BL, J], dt, tag="chc")
            nc.vector.tensor_tensor(a_[:], cmv["h11"][:, g_, :], X1g[:, g_, 0, :], op=mult)
            nc.vector.tensor_tensor(b_[:], cmv["h12"][:, g_, :], X2g[:, g_, 0, :], op=mult)
            nc.vector.tensor_tensor(c_[:], a_[:], b_[:], op=add)
            nc.vector.tensor_tensor(X1g[:, g_ + 1, 0, :], c_[:], cmv["p1"][:, g_, :], op=add)
            a2_ = tpool.tile([BL, J], dt, tag="cha")
            b2_ = tpool.tile([BL, J], dt, tag="chb")
            c2_ = tpool.tile([BL, J], dt, tag="chc")
            nc.vector.tensor_tensor(a2_[:], cmv["h21"][:, g_, :], X1g[:, g_, 0, :], op=mult)
            nc.vector.tensor_tensor(b2_[:], cmv["h22"][:, g_, :], X2g[:, g_, 0, :], op=mult)
            nc.vector.tensor_tensor(c2_[:], a2_[:], b2_[:], op=add)
            nc.vector.tensor_tensor(X2g[:, g_ + 1, 0, :], c2_[:], cmv["p2"][:, g_, :], op=add)

        # recover within-group inits: X_{g,l} = link_{l-1}(X_{g,l-1}), vec over g
        for l in range(1, LNK):
            lnk = {k: Hg[k][:, :, l - 1, :] for k in H}
            a_ = tpool.tile([BL, GRP * J], dt, tag="rca")
            av = a_[:].rearrange("p (g j) -> p g j", j=J)
            b_ = tpool.tile([BL, GRP * J], dt, tag="rcb")
            bv = b_[:].rearrange("p (g j) -> p g j", j=J)
            nc.vector.tensor_tensor(av, lnk["h11"], X1g[:, :, l - 1, :], op=mult)
            nc.vector.tensor_tensor(bv, lnk["h12"], X2g[:, :, l - 1, :], op=mult)
            e_ = tpool.tile([BL, GRP * J], dt, tag="rce")
            ev2 = e_[:].rearrange("p (g j) -> p g j", j=J)
            nc.vector.tensor_tensor(ev2, av, bv, op=add)
            c_ = tpool.tile([BL, GRP * J], dt, tag="rcc")
            cv2 = c_[:].rearrange("p (g j) -> p g j", j=J)
            d_ = tpool.tile([BL, GRP * J], dt, tag="rcd")
            dv2 = d_[:].rearrange("p (g j) -> p g j", j=J)
            f_ = tpool.tile([BL, GRP * J], dt, tag="rcf")
            fv2 = f_[:].rearrange("p (g j) -> p g j", j=J)
            gps.tensor_tensor(cv2, lnk["h21"], X1g[:, :, l - 1, :], op=mult)
            gps.tensor_tensor(dv2, lnk["h22"], X2g[:, :, l - 1, :], op=mult)
            gps.tensor_tensor(fv2, cv2, dv2, op=add)
            nc.vector.tensor_tensor(X1g[:, :, l, :], ev2, lnk["p1"], op=add)
            gps.tensor_tensor(X2g[:, :, l, :], fv2, lnk["p2"], op=add)

        # ---------------- pass C ----------------
        # th threaded as [p, c, j] APs; th' written straight into the
        # just-consumed U slice, which turns U_b into the output buffer.
        th_ap = X1i[:].rearrange("p (c j) -> p c j", j=J)
        vt_ap = X2i[:].rearrange("p (c j) -> p c j", j=J)
        outv = U_b[:].rearrange("p (c s j) -> p c s j", s=C, j=J)
        for s in range(C):
            g = nGs[:, :, s, :]
            mm = tpool.tile([BL, FREE], dt, tag="mmP")
            (gps if gps_passc else nc.vector).tensor_tensor(cv(mm), g, th_ap, op=mult)
            vd = tpool.tile([BL, FREE], dt, tag="vdP")
            gps.tensor_tensor(cv(vd), d3v, vt_ap, op=mult)
            vt_n = spool.tile([BL, FREE], dt, tag="thB")
            nc.vector.tensor_tensor(vt_n[:], vd[:], mm[:], op=add)
            vt2 = spool.tile([BL, FREE], dt, tag="vtB")
            nc.vector.tensor_tensor(cv(vt2), cv(vt_n), Us[:, :, s, :], op=add)
            th_n = spool.tile([BL, FREE], dt, tag="thA", name="thO")
            nc.vector.tensor_tensor(cv(th_n), th_ap, cv(vt2), op=add)
            nc.sync.dma_start(out_d[:, s * FREE:(s + 1) * FREE], th_n[:])
            th_ap = cv(th_n)
            vt_ap = cv(vt2)

        # final dtheta: vt of last chunk * SR
        vt_last = vt_ap[:, n_c - 1, :]
        vts = tpool.tile([BL, J], dt, tag="vts")
        nc.vector.tensor_scalar_mul(vts[:], vt_last, float(SR))
        nc.sync.dma_start(vt_d, vts[:])


    nc.compile()
    return nc


# ---------------------------------------------------------------------------
# host glue
# ---------------------------------------------------------------------------

def _host_weights(M, inertia, damping, T, n_c, C, tblk):
    d64 = 1.0 - DT * damping.astype(np.float64) / inertia.astype(np.float64)
    DT2_I = DT * DT / inertia.astype(np.float64)
    M64 = M.astype(np.float64)
    wf = (DT2_I[:, None] * M64).astype(f32)          # [J,2]
    nwk = (-DT2_I[:, None] * M64 ** 2).astype(f32)   # [J,2]
    d_ = d64.astype(f32)

    def rep(vecj, n):
        return np.broadcast_to(np.tile(vecj, n)[None, :], (BL, n * J)).copy()

    return {
        "wf0": rep(wf[:, 0], tblk),
        "wf1": rep(wf[:, 1], tblk),
        "wk0n": rep(nwk[:, 0], tblk),
        "wk1n": rep(nwk[:, 1], tblk),
        "d3": rep(d_, n_c),
    }


_CACHE = {}


def _get_kernel(T=2048, n_c=64, C=32, tblk=64):
    key = (T, n_c, C, tblk)
    if key not in _CACHE:
        _CACHE[key] = build_kernel(T, n_c, C, tblk)
    return _CACHE[key]


def kernel(x, state0, M, inertia, damping):
    from concourse import bass_utils

    x = np.ascontiguousarray(x, dtype=f32)
    state0 = np.ascontiguousarray(state0, dtype=f32)
    M = np.asarray(M, f32)
    inertia = np.asarray(inertia, f32)
    damping = np.asarray(damping, f32)

    B, T, _ = x.shape
    n_c, C, tblk = 64, 32, 64
    nc = _get_kernel(T, n_c, C, tblk)
    w = _host_weights(M, inertia, damping, T, n_c, C, tblk)

    in_maps = []
    for k in range(N_CORES):
        sl = slice(k * BL, (k + 1) * BL)
        in_maps.append({
            "xs": x[sl].reshape(BL, T * 32),
            "st_th": np.ascontiguousarray(state0[sl, :, 0]),
            "st_vt": np.ascontiguousarray(state0[sl, :, 1] * f32(DT)),
            **w,
        })

    res = bass_utils.run_bass_kernel_spmd(nc, in_maps, core_ids=list(range(N_CORES)))

    out = np.empty((B, T, J), f32)
    v_T = np.empty((B, J), f32)
    for k in range(N_CORES):
        sl = slice(k * BL, (k + 1) * BL)
        out[sl] = (res.results[k]["out_d"]
                   .reshape(BL, C, n_c, J).transpose(0, 2, 1, 3).reshape(BL, T, J))
        v_T[sl] = res.results[k]["vt_d"]

    th_T = out[:, T - 1, :]
    final_state = np.stack([th_T, v_T], axis=2)
    muscle_states = final_state[..., None] * M[None, :, None, :]
    return out, muscle_states, final_state
